# revision 26
# baseline (speedup 1.0000x reference)
"""Trainium2 Bass kernel for AdaptiveScaledDotProductAttention.

Sharding: DP=4 over batch x TP=2 over heads (8 NeuronCores).
Core c handles batch c//2, head-group g=c%2 (heads 8g..8g+7).
Each core projects q/k/v/s for its 8 heads over the full sequence,
runs attention, and computes a PARTIAL output projection against the
full Wo (its 8 heads' rows). The host sums the two partials per batch
during the unshard/gather step -- no on-chip collective at all.

Layout strategy:
 - Host passes inputs pre-transposed (feature-major x^T) and pre-cast
   to bf16 (identical numerics to the baseline's on-chip cast; removes
   all PE transposes of inputs and halves input DMA).
 - q/k/s projected feature-major with HEAD PAIRS stacked across the
   128 partitions -> QK runs as two concurrent 64-row-tiled matmuls
   (tile_position auto-derived from base partition 0 / 64), keeping
   the full PE array active.
 - AV uses E (exp scores) as the STATIONARY operand and V (+ ones
   column for the softmax denominator) as moving -> full 128x128
   array, token-major output [q, d] in PSUM.
 - Token-major attention output makes the softmax division a native
   DVE per-partition tensor_scalar op (no PE broadcast tricks).
 - The per-query language logit rides into column 65 of the same PSUM
   accumulator via a tiny K=64 matmul over p = q*s.
 - att is PE-transposed back to feature-major (32 cheap 128x128
   transposes) for the output projection.
"""

import numpy as np

H, DK, DV, DM = 16, 64, 64, 1024
B, N = 4, 1024
SCALE = float(1.0 / np.sqrt(DK))
NCORES = 8
HLOC = 8          # heads per core
NPAIR = HLOC // 2  # head pairs per core
FLOC = HLOC * DK  # 512 local features

_CACHE = {}
DEBUG_TAPS = False
K_ITER = 1  # >1: loop whole kernel in-graph (timing only)


def _build(with_biases, k_iter=1):
    import concourse.bass as bass
    import concourse.tile as tile
    from concourse import bacc, mybir
    from concourse.masks import make_identity

    f32 = mybir.dt.float32
    bf16 = mybir.dt.bfloat16
    Exp = mybir.ActivationFunctionType.Exp
    Copy = mybir.ActivationFunctionType.Copy

    nc = bacc.Bacc("TRN2", target_bir_lowering=False, debug=False,
                   num_devices=NCORES)

    def din(name, shape, dt=bf16):
        return nc.dram_tensor(name, shape, dt, kind="ExternalInput").ap()

    # feature-major (transposed) activations, bf16, staged on host
    xq = din("xqT", [DM, N])
    xk = din("xkT", [DM, N])
    xv = din("xvT", [DM, N])
    xs = din("xsT", [DM, N])
    wq = din("wq", [DM, FLOC])
    wk = din("wk", [DM, FLOC])
    wv = din("wv", [DM, FLOC])
    ws = din("ws", [DM, FLOC])
    wo = din("wo", [FLOC, DM])      # local heads' rows of Wo, full dm
    bqp = din("bqp", [128, NPAIR], f32)   # pair-major per-partition bias
    bkp = din("bkp", [128, NPAIR], f32)
    bsp = din("bsp", [128, NPAIR], f32)
    bvr = din("bvr", [1, FLOC], f32)      # row biases for token-major v/s
    bsr = din("bsr", [1, FLOC], f32)
    out = nc.dram_tensor("out", [N, DM], bf16, kind="ExternalOutput").ap()
    dbg = {}
    if DEBUG_TAPS:
        for nm, shp, dt_ in (
                ("d_qT2", [128, NPAIR, N], "bf16"),
                ("d_kT2", [128, NPAIR, N], "bf16"),
                ("d_sT2", [128, NPAIR, N], "bf16"),
                ("d_stok", [128, 8, FLOC], "bf16"),
                ("d_vaug", [128, 8, HLOC, DV + 1], "bf16"),
                ("d_E0", [128, 2, 512], "bf16"),
                ("d_av0", [128, 2, 512], "f32"),
                ("d_att", [128, 8, FLOC], "f32"),
                ("d_attf", [128, 4, N], "bf16")):
            dd = bf16 if dt_ == "bf16" else f32
            dbg[nm] = nc.dram_tensor(nm, shp, dd, kind="ExternalOutput").ap()

    from contextlib import ExitStack
    with ExitStack() as top:
        tc = top.enter_context(tile.TileContext(nc))

        persist = top.enter_context(tc.tile_pool(name="persist", bufs=1))
        # inputs (feature-major planes) + weights
        xq_sb = persist.tile([128, 8, N], bf16)
        xk_sb = persist.tile([128, 8, N], bf16)
        xv_sb = persist.tile([128, 8, N], bf16)
        xs_sb = persist.tile([128, 8, N], bf16)
        wq_sb = persist.tile([128, 8, FLOC], bf16)
        wk_sb = persist.tile([128, 8, FLOC], bf16)
        wv_sb = persist.tile([128, 8, FLOC], bf16)
        ws_sb = persist.tile([128, 8, FLOC], bf16)
        wo_sb = persist.tile([128, 4, DM], bf16)
        # projections: head-pair-stacked feature-major, token-major v/s
        qT2 = persist.tile([128, NPAIR, N], bf16)
        kT2 = persist.tile([128, NPAIR, N], bf16)
        sT2 = persist.tile([128, NPAIR, N], bf16)
        s_tok = persist.tile([128, 8, FLOC], bf16)
        vaug = persist.tile([128, 8, HLOC, DV + 1], bf16)
        att_tok = persist.tile([128, 8, FLOC], f32)
        att_feat = persist.tile([128, 4, N], bf16)
        identity = persist.tile([128, 128], bf16)
        identity_f = persist.tile([128, 128], f32)
        ones = persist.tile([128, 1], bf16)
        ones_row = persist.tile([1, 128], bf16)

        make_identity(nc, identity)
        nc.vector.tensor_copy(out=identity_f, in_=identity)
        nc.vector.memset(ones[:, :], 1.0)
        nc.vector.memset(ones_row[:, :], 1.0)
        nc.vector.memset(vaug[:, :, :, DV:DV + 1], 1.0)

        if with_biases:
            bq_sb = persist.tile([128, NPAIR], f32)
            bk_sb = persist.tile([128, NPAIR], f32)
            bs_sb = persist.tile([128, NPAIR], f32)
            nc.sync.dma_start(out=bq_sb, in_=bqp)
            nc.sync.dma_start(out=bk_sb, in_=bkp)
            nc.sync.dma_start(out=bs_sb, in_=bsp)
            bvrow = persist.tile([1, FLOC], f32)
            bsrow = persist.tile([1, FLOC], f32)
            nc.sync.dma_start(out=bvrow, in_=bvr)
            nc.sync.dma_start(out=bsrow, in_=bsr)

        # Stream inputs in consumption order. DMA issue costs ~650ns per
        # instruction on the sync queue and all queues share HBM BW, so:
        # interleave x/w planes for q/k (2-plane granularity, consumed
        # immediately) and coarsen the later tensors into halves.
        def load_chunks(dst, src, nplane, group):
            ap = src.rearrange("(j p) c -> p j c", p=128)
            for j0 in range(0, nplane, group):
                nc.sync.dma_start(out=dst[:, j0:j0 + group, :],
                                  in_=ap[:, j0:j0 + group, :])

        def _pc(dst, src, j0, j1):
            nc.sync.dma_start(
                out=dst[:, j0:j1, :],
                in_=src.rearrange("(j p) c -> p j c", p=128)[:, j0:j1, :])

        # half/single-plane first chunks so the very first matmul can
        # start as early as possible (proj half 0 reads cols 0:512 only)
        nc.sync.dma_start(
            out=xq_sb[:, 0:1, 0:512],
            in_=xq.rearrange("(j p) c -> p j c", p=128)[:, 0:1, 0:512])
        _pc(wq_sb, wq, 0, 1)
        nc.sync.dma_start(
            out=xq_sb[:, 0:1, 512:1024],
            in_=xq.rearrange("(j p) c -> p j c", p=128)[:, 0:1, 512:1024])
        _pc(xq_sb, xq, 1, 2)
        _pc(wq_sb, wq, 1, 2)
        for j0 in range(2, 8, 2):
            _pc(xq_sb, xq, j0, j0 + 2)
            _pc(wq_sb, wq, j0, j0 + 2)
        for j0 in range(0, 8, 2):
            nc.sync.dma_start(
                out=xk_sb[:, j0:j0 + 2, :],
                in_=xk.rearrange("(j p) c -> p j c", p=128)[:, j0:j0 + 2, :])
            nc.sync.dma_start(
                out=wk_sb[:, j0:j0 + 2, :],
                in_=wk.rearrange("(j p) c -> p j c", p=128)[:, j0:j0 + 2, :])
        load_chunks(xv_sb, xv, 8, 4)
        load_chunks(wv_sb, wv, 8, 4)
        load_chunks(xs_sb, xs, 8, 4)
        load_chunks(ws_sb, ws, 8, 4)
        load_chunks(wo_sb, wo, 4, 2)

        av_dbg_sb = None
        if DEBUG_TAPS:
            av_dbg_sb = persist.tile([128, 2, 512], f32, name="av_dbg_sb")
        ppool = top.enter_context(tc.tile_pool(name="ppool", bufs=2))
        epool = top.enter_context(tc.tile_pool(name="epool", bufs=10))
        smpool = top.enter_context(tc.tile_pool(name="smpool", bufs=2))
        w2pool = top.enter_context(tc.tile_pool(name="w2pool", bufs=4))
        stpool = top.enter_context(tc.tile_pool(name="stpool", bufs=3))
        ps_o = top.enter_context(
            tc.tile_pool(name="ps_o", bufs=2, space="PSUM"))
        ps_sc = top.enter_context(
            tc.tile_pool(name="ps_sc", bufs=2, space="PSUM"))
        ps_av = top.enter_context(
            tc.tile_pool(name="ps_av", bufs=1, space="PSUM"))

        if with_biases:
            # token-major bias planes built once via K=1 matmul broadcast
            bvb = persist.tile([1, FLOC], bf16)
            bsb = persist.tile([1, FLOC], bf16)
            nc.vector.tensor_copy(out=bvb, in_=bvrow)
            nc.vector.tensor_copy(out=bsb, in_=bsrow)
            ps = ps_o.tile([128, FLOC], f32, tag="ps_proj")
            nc.tensor.matmul(ps, ones_row, bvb, start=True, stop=True)
            bv_plane = persist.tile([128, FLOC], f32)
            nc.vector.tensor_copy(out=bv_plane, in_=ps)
            ps = ps_o.tile([128, FLOC], f32, tag="ps_proj")
            nc.tensor.matmul(ps, ones_row, bsb, start=True, stop=True)
            bs_plane = persist.tile([128, FLOC], f32)
            nc.vector.tensor_copy(out=bs_plane, in_=ps)

        for _it in range(k_iter):
            # feature-major projection of one head pair (q/k/s)
            def proj_pair(x_sb, w_sb, t, dst, bias):
                for half in range(2):
                    ps = ps_o.tile([128, 512], f32, tag="ps_proj")
                    for j in range(8):
                        nc.tensor.matmul(
                            ps, w_sb[:, j, t * 128:(t + 1) * 128],
                            x_sb[:, j, half * 512:(half + 1) * 512],
                            start=(j == 0), stop=(j == 7))
                    dsl = dst[:, t, half * 512:(half + 1) * 512]
                    nc.vector.tensor_copy(out=dsl, in_=ps)
                    if with_biases:
                        nc.vector.tensor_scalar_add(dsl, dsl, bias[:, t:t + 1])

            # token-major projection (v / s_tok), one token block
            def proj_tok(x_sb, w_sb, tb, evac):
                ps = ps_o.tile([128, 512], f32, tag="ps_proj")
                for j in range(8):
                    nc.tensor.matmul(
                        ps, x_sb[:, j, tb * 128:(tb + 1) * 128],
                        w_sb[:, j, :], start=(j == 0), stop=(j == 7))
                evac(ps, tb)

            def evac_v(ps, tb):
                if with_biases:
                    t2 = stpool.tile([128, FLOC], f32, tag="bias_tmp")
                    nc.vector.tensor_add(t2, ps, bv_plane)
                    nc.vector.tensor_copy(
                        out=vaug[:, tb, :, 0:DV],
                        in_=t2.rearrange("p (h d) -> p h d", h=HLOC))
                else:
                    nc.vector.tensor_copy(
                        out=vaug[:, tb, :, 0:DV],
                        in_=ps.rearrange("p (h d) -> p h d", h=HLOC))

            def evac_stok(ps, tb):
                if with_biases:
                    t2 = stpool.tile([128, FLOC], f32, tag="bias_tmp")
                    nc.vector.tensor_add(t2, ps, bs_plane)
                    nc.vector.tensor_copy(out=s_tok[:, tb, :], in_=t2)
                else:
                    nc.vector.tensor_copy(out=s_tok[:, tb, :], in_=ps)

            # phase C: transpose att to feature-major + partial out proj
            def phase_c(tbs):
                for tb in tbs:
                    ps = ps_o.tile([128, 512], f32, tag="ps_proj")
                    pack = ps.rearrange("p (a b) -> p a b", a=4)
                    for fc in range(4):
                        nc.tensor.transpose(
                            pack[:, fc, :],
                            att_tok[:, tb, fc * 128:(fc + 1) * 128],
                            identity_f)
                    nc.vector.tensor_copy(
                        out=att_feat[:, :, tb * 128:(tb + 1) * 128],
                        in_=pack)
                for tb in tbs:
                    for half in range(2):
                        po = ps_o.tile([128, 512], f32, tag="ps_proj")
                        for fc in range(4):
                            nc.tensor.matmul(
                                po,
                                att_feat[:, fc, tb * 128:(tb + 1) * 128],
                                wo_sb[:, fc, half * 512:(half + 1) * 512],
                                start=(fc == 0), stop=(fc == 3))
                        ost = stpool.tile([128, 512], bf16, tag="ostage")
                        nc.vector.tensor_copy(out=ost, in_=po)
                        nc.sync.dma_start(
                            out=out[tb * 128:(tb + 1) * 128,
                                    half * 512:(half + 1) * 512],
                            in_=ost)

            # ---- software-pipelined emission schedule ----
            # Attention "units" (pair t, query half qc) emit QK chunk
            # groups interleaved with ~1.7us projection filler pieces so
            # the in-order PE queue never idles while the scalar engine
            # works through the exps that gate AV.
            def proj_half(x_sb, w_sb, t, dst, bias, half):
                ps = ps_o.tile([128, 512], f32, tag="ps_proj")
                for j in range(8):
                    nc.tensor.matmul(
                        ps, w_sb[:, j, t * 128:(t + 1) * 128],
                        x_sb[:, j, half * 512:(half + 1) * 512],
                        start=(j == 0), stop=(j == 7))
                dsl = dst[:, t, half * 512:(half + 1) * 512]
                nc.vector.tensor_copy(out=dsl, in_=ps)
                if with_biases:
                    nc.vector.tensor_scalar_add(dsl, dsl, bias[:, t:t + 1])

            p_map = {}
            w2_saved = {}
            mult = mybir.AluOpType.mult
            add = mybir.AluOpType.add

            def add_s(t):
                # deferred s-term for qc=0 tokens: att += s_tok * w2
                def f():
                    w2 = w2_saved[t]
                    for h2 in range(2):
                        h = 2 * t + h2
                        for qb in range(4):
                            sl = att_tok[:, qb, h * DV:(h + 1) * DV]
                            nc.vector.scalar_tensor_tensor(
                                sl, s_tok[:, qb, h * DV:(h + 1) * DV],
                                w2[:, h2, qb:qb + 1], sl, mult, add)
                return f

            def mk_p(t):
                def f():
                    p = ppool.tile([128, N], bf16, tag="p")
                    nc.vector.tensor_mul(p, qT2[:, t, :], sT2[:, t, :])
                    p_map[t] = p
                return f

            bq = bq_sb if with_biases else None
            bk = bk_sb if with_biases else None
            bs = bs_sb if with_biases else None
            Qf = lambda t, h: (lambda: proj_half(xq_sb, wq_sb, t, qT2, bq, h))
            Kf = lambda t, h: (lambda: proj_half(xk_sb, wk_sb, t, kT2, bk, h))
            Sf = lambda t, h: (lambda: proj_half(xs_sb, ws_sb, t, sT2, bs, h))
            Vf = lambda tb: (lambda: proj_tok(xv_sb, wv_sb, tb, evac_v))
            STf = lambda tb: (lambda: proj_tok(xs_sb, ws_sb, tb, evac_stok))
            PCf = lambda tb: (lambda: phase_c([tb]))

            def att_unit(t, qc, fillers, extras):
                qs = slice(qc * 512, (qc + 1) * 512)
                Es = []
                nf = 0
                for g in range(4):
                    for kb in (2 * g, 2 * g + 1):
                        sc = ps_sc.tile([128, 2, 512], f32, tag="sc")
                        for h2 in range(2):
                            hp = slice(h2 * 64, (h2 + 1) * 64)
                            nc.tensor.matmul(
                                sc[:, h2, :],
                                kT2[hp, t, kb * 128:(kb + 1) * 128],
                                qT2[hp, t, qs],
                                start=True, stop=True)
                        E = epool.tile([128, 2, 512], bf16, tag="E")
                        nc.scalar.activation(
                            E.rearrange("p a b -> p (a b)"),
                            sc.rearrange("p a b -> p (a b)"),
                            Exp, scale=SCALE)
                        Es.append(E)
                    if nf < len(fillers):
                        fillers[nf]()
                        nf += 1
                while nf < len(fillers):
                    fillers[nf]()
                    nf += 1

                p = p_map[t]
                av = ps_av.tile([128, 2, 512], f32, tag="av")
                # language logits ride as column 65 of each q-block
                for h2 in range(2):
                    hp = slice(h2 * 64, (h2 + 1) * 64)
                    for qb in range(4):
                        nc.tensor.matmul(
                            av[:, h2, qb * 66 + 65:qb * 66 + 66],
                            p[hp, qc * 512 + qb * 128:
                              qc * 512 + (qb + 1) * 128],
                            ones[hp, :],
                            start=True, stop=True)
                # AV: E stationary (full 128x128), vaug+ones moving.
                # NOTE: each (h2, qb) accumulation group must run to
                # completion before the next group's START in the same
                # PSUM bank -- START clears has_written coarsely, which
                # turns interleaved groups' accumulates into overwrites.
                for qb in range(4):
                    for h2 in range(2):
                        for kb in range(8):
                            nc.tensor.matmul(
                                av[:, h2, qb * 66:qb * 66 + 65],
                                Es[kb][:, h2, qb * 128:(qb + 1) * 128],
                                vaug[:, kb, 2 * t + h2, :],
                                start=(kb == 0), stop=(kb == 7))

                for f in extras:
                    f()

                # softmax epilogue, token-major. Column views of the
                # packed av layout: [128, h2, qb, 66] -> col c
                avq = av[:, :, 0:4 * 66].rearrange(
                    "p a (q c) -> p a q c", q=4)

                def av_col(c):
                    return avq[:, :, :, c:c + 1].rearrange(
                        "p a q c -> p a (q c)")

                d0 = smpool.tile([128, 2, 4], f32, tag="d0")
                l0 = smpool.tile([128, 2, 4], f32, tag="l0")
                nc.vector.tensor_copy(out=d0, in_=av_col(DV))
                nc.vector.tensor_copy(out=l0, in_=av_col(DV + 1))
                el = smpool.tile([128, 2, 4], f32, tag="el")
                nc.scalar.activation(el, l0, Exp, scale=SCALE)
                den = smpool.tile([128, 2, 4], f32, tag="den")
                nc.vector.tensor_add(den, d0, el)
                rc = smpool.tile([128, 2, 4], f32, tag="rc")
                nc.vector.reciprocal(rc, den)
                if qc == 0:
                    w2 = w2pool.tile([128, 2, 4], f32, tag="w2d")
                    w2_saved[t] = w2
                else:
                    w2 = smpool.tile([128, 2, 4], f32, tag="w2")
                nc.vector.tensor_mul(w2, el, rc)

                for h2 in range(2):
                    h = 2 * t + h2
                    for qb in range(4):
                        tb = qc * 4 + qb
                        if qc == 0:
                            # s-term deferred to add_s(t) later
                            nc.vector.tensor_scalar_mul(
                                att_tok[:, tb, h * DV:(h + 1) * DV],
                                av[:, h2, qb * 66:qb * 66 + 64],
                                rc[:, h2, qb:qb + 1])
                        else:
                            tmp = stpool.tile([128, DV], f32, tag="tmp")
                            nc.vector.tensor_scalar_mul(
                                tmp, s_tok[:, tb, h * DV:(h + 1) * DV],
                                w2[:, h2, qb:qb + 1])
                            nc.vector.scalar_tensor_tensor(
                                att_tok[:, tb, h * DV:(h + 1) * DV],
                                av[:, h2, qb * 66:qb * 66 + 64],
                                rc[:, h2, qb:qb + 1],
                                tmp, mult, add)

                if DEBUG_TAPS and t == 0 and qc == 0:
                    nc.sync.dma_start(out=dbg["d_E0"], in_=Es[0])
                    nc.vector.tensor_copy(out=av_dbg_sb, in_=av)
                    nc.sync.dma_start(out=dbg["d_av0"], in_=av_dbg_sb)

            # prologue: q/k of pair 0 (first DMA arrivals)
            for h in range(2):
                Qf(0, h)()
            for h in range(2):
                Kf(0, h)()

            schedule = [
                (0, 0, [Vf(0), Vf(1), Vf(2), Vf(3), Vf(4), Vf(5), Vf(6),
                        Vf(7), Sf(0, 0), Sf(0, 1), mk_p(0)], []),
                (0, 1, [Qf(1, 0), Qf(1, 1), Kf(1, 0), Kf(1, 1)],
                       [STf(4), STf(5), STf(6), STf(7)]),
                (1, 0, [Sf(1, 0), Sf(1, 1), Sf(2, 0), Sf(2, 1), mk_p(1)],
                       []),
                (1, 1, [Qf(2, 0), Qf(2, 1), Kf(2, 0), Kf(2, 1), STf(0)],
                       []),
                (2, 0, [Sf(3, 0), Sf(3, 1), Qf(3, 0), Qf(3, 1), mk_p(2),
                        STf(1)], []),
                (2, 1, [Kf(3, 0), Kf(3, 1), STf(2), STf(3), add_s(0)],
                       []),
                (3, 0, [mk_p(3), add_s(1), add_s(2)], []),
                (3, 1, [add_s(3), PCf(0), PCf(1), PCf(2), PCf(3)], []),
            ]
            for t, qc, fillers, extras in schedule:
                att_unit(t, qc, fillers, extras)
            phase_c(range(4, 8))
            if DEBUG_TAPS:
                nc.sync.dma_start(out=dbg["d_qT2"], in_=qT2)
                nc.sync.dma_start(out=dbg["d_kT2"], in_=kT2)
                nc.sync.dma_start(out=dbg["d_sT2"], in_=sT2)
                nc.sync.dma_start(out=dbg["d_stok"], in_=s_tok)
                nc.sync.dma_start(out=dbg["d_vaug"], in_=vaug)
                nc.sync.dma_start(out=dbg["d_att"], in_=att_tok)
                nc.sync.dma_start(out=dbg["d_attf"], in_=att_feat)

    nc.compile()
    return nc


def _get_nc(with_biases):
    key = ("nc", with_biases, K_ITER)
    if key not in _CACHE:
        _CACHE[key] = _build(with_biases, K_ITER)
    return _CACHE[key]


def kernel(queries, keys, values, language_signals,
           Wq, b_q, Wk, b_k, Wv, b_v, Ws, b_s, Wo, b_o):
    from concourse.bass_utils import run_bass_kernel_spmd
    import ml_dtypes

    bf = ml_dtypes.bfloat16
    with_biases = any(
        np.any(np.asarray(b)) for b in (b_q, b_k, b_v, b_s, b_o))
    nc = _get_nc(with_biases)

    def bias_pairs(b, hs):
        # [512] feature bias -> [128, 4] pair-major per-partition layout
        return np.ascontiguousarray(
            np.asarray(b[hs], np.float32).reshape(4, 128).T)

    in_maps = []
    for core in range(NCORES):
        b, g = core // 2, core % 2
        hs = slice(FLOC * g, FLOC * (g + 1))
        in_maps.append({
            "xqT": np.ascontiguousarray(np.asarray(queries[b]).T, dtype=bf),
            "xkT": np.ascontiguousarray(np.asarray(keys[b]).T, dtype=bf),
            "xvT": np.ascontiguousarray(np.asarray(values[b]).T, dtype=bf),
            "xsT": np.ascontiguousarray(
                np.asarray(language_signals[b]).T, dtype=bf),
            "wq": np.ascontiguousarray(Wq[:, hs], dtype=bf),
            "wk": np.ascontiguousarray(Wk[:, hs], dtype=bf),
            "wv": np.ascontiguousarray(Wv[:, hs], dtype=bf),
            "ws": np.ascontiguousarray(Ws[:, hs], dtype=bf),
            "wo": np.ascontiguousarray(Wo[hs, :], dtype=bf),
            "bqp": bias_pairs(b_q, hs),
            "bkp": bias_pairs(b_k, hs),
            "bsp": bias_pairs(b_s, hs),
            "bvr": np.ascontiguousarray(
                np.asarray(b_v[hs], np.float32).reshape(1, -1)),
            "bsr": np.ascontiguousarray(
                np.asarray(b_s[hs], np.float32).reshape(1, -1)),
        })
    _CACHE["last_in_maps"] = in_maps
    res = run_bass_kernel_spmd(nc, in_maps, list(range(NCORES))).results
    full = np.empty((B, N, DM), np.float32)
    for b in range(B):
        full[b] = (np.asarray(res[2 * b]["out"], np.float32)
                   + np.asarray(res[2 * b + 1]["out"], np.float32))
    full += np.asarray(b_o, np.float32)
    return full
